# revision 31
# baseline (speedup 1.0000x reference)
"""GC-LSTM (graph-conv LSTM) Trainium2 kernel, v2.

Model (per batch b, timestep t):
    msg  = relu([x_t, h] @ Wg + bg)          # [N, H]
    agg  = adj @ msg                         # graph aggregation over stations
    gates= agg @ Wl + bl                     # [N, 4H]
    c    = sig(f)*c + sig(i)*tanh(g)
    h    = sig(o)*tanh(c)
final: out = relu(h[:, 0, :] @ Wr1 + br1) @ Wr2 + br2     # [B, 1]

Sharding: data-parallel over B=32 across 8 cores (B_local=4). adj and
weights replicated. h/c stay on-core through the whole T=168 scan.

Per-core layouts (b = 2*pair + q):
  SL (station-layout): partition = station-within-ktile.
    msg_ps  [128, (kt,b,h)]   PSUM, col = kt*256 + b*64 + h
    msg_sb  [128, (pair,kt,qh)] SBUF, col = pair*512 + kt*128 + q*64 + h
    adjT_sb [128, (kt,m)]     adjT_sb[p, kt*512+m] = adj[m, kt*128+p]
  PP (parity-paired feature-layout): partition = (q, h), free = (pair, m).
    hT2, cT2 [128, 1024] fp16
    agg_ps/gate_ps per pair: [128 (q,h'), m] with gate order (i, f, o, g)

Key tricks vs v1:
  - batch lives in the matmul CONTRACTION dim with block-diagonal weights
    (wgx_blk4 [36,256] for 4 batches; wgh2_blk / wl_blk [128,128] for the
    2 parities), collapsing many tiny matmuls into few wide ones.
  - relu and agg fp32->fp16 casts run on the idle Pool (gpsimd) engine.
  - c state is fp16 so every LSTM element-wise op is 16-bit SBUF-only
    (DVE 2x/4x perf modes).
"""

import os
import sys
import types
from contextlib import ExitStack

import numpy as np

import concourse.bass as bass
import concourse.mybir as mybir
import concourse.tile as tile
from concourse import bacc
from concourse.bass_utils import run_bass_kernel_spmd


def _ensure_ntff_hook():
    """Register the axon NTFF profile hook if the image's antenv lacks it."""
    try:
        from antenv import axon_hooks  # noqa: F401
        return
    except ImportError:
        pass
    hook = None
    try:
        import importlib
        tb = importlib.import_module("trn_agent_boot.trn_boot")
        hook = tb._ntff_profile_via_ctypes("/opt/axon/libaxon_pjrt.so")
    except Exception:
        hook = None
    mod = types.ModuleType("antenv.axon_hooks")
    mod._hook = hook
    mod.set_axon_ntff_profile_hook = lambda h: setattr(mod, "_hook", h)
    mod.get_axon_ntff_profile_hook = lambda: mod._hook
    import antenv
    sys.modules["antenv.axon_hooks"] = mod
    antenv.axon_hooks = mod

AF = mybir.ActivationFunctionType
DT = mybir.dt

# Problem constants (hardcoded per contract)
B, N, T, F = 32, 512, 168, 8
H = 64
NCORES = 8
BL = B // NCORES          # 4 batches per core
NPAIR = BL // 2           # 2 pairs
KT = N // 128             # 4 station k-tiles
FR = F + 1                # x rows incl. ones row
XR = BL * FR              # 36 x rows for all 4 batches
TC = 24                   # x prefetch chunk (168 = 7*24)


def build_program(t_steps: int = T, n_devices: int = NCORES):
    """Build the SPMD Bass program for one core (replicated on all cores)."""
    nc = bacc.Bacc("TRN2", target_bir_lowering=False, debug=False,
                   num_devices=n_devices)

    fp32 = DT.float32
    f16 = DT.float16
    f8 = DT.float8e4

    # DRAM I/O (per-core values supplied via in_maps)
    xT4_d = nc.dram_tensor("xT4", [XR, t_steps, N], f16, kind="ExternalInput").ap()
    adjT_d = nc.dram_tensor("adjT", [128, KT * N], f16, kind="ExternalInput").ap()
    wgx4_d = nc.dram_tensor("wgx4", [XR, BL * H], f16, kind="ExternalInput").ap()
    wgh2_d = nc.dram_tensor("wgh2b", [128, 128], f16, kind="ExternalInput").ap()
    wlblk_d = nc.dram_tensor("wlblk", [128, 4 * 128], f16, kind="ExternalInput").ap()
    wr12_d = nc.dram_tensor("wr12", [128, H // 2], f16, kind="ExternalInput").ap()
    br1_d = nc.dram_tensor("br1", [H // 2, 1], fp32, kind="ExternalInput").ap()
    wr2_d = nc.dram_tensor("wr2", [H // 2, 1], f16, kind="ExternalInput").ap()
    br2_d = nc.dram_tensor("br2", [1, 1], fp32, kind="ExternalInput").ap()
    out_d = nc.dram_tensor("out", [1, BL], fp32, kind="ExternalOutput").ap()

    with tile.TileContext(nc) as tc, ExitStack() as ctx:
        const_pool = ctx.enter_context(tc.tile_pool(name="const", bufs=1))
        state_pool = ctx.enter_context(tc.tile_pool(name="state", bufs=1))
        xc_pool = ctx.enter_context(tc.tile_pool(name="xc", bufs=2))
        msgsb_pool = ctx.enter_context(tc.tile_pool(name="msgsb", bufs=2))
        aggsb_pool = ctx.enter_context(tc.tile_pool(name="aggsb", bufs=3))
        gact_pool = ctx.enter_context(tc.tile_pool(name="gact", bufs=3))
        tmp_pool = ctx.enter_context(tc.tile_pool(name="tmp", bufs=6))

        pp_msg = ctx.enter_context(tc.tile_pool(name="pp_msg", bufs=1, space="PSUM"))
        pp_agg = ctx.enter_context(tc.tile_pool(name="pp_agg", bufs=2, space="PSUM"))
        pp_gate = ctx.enter_context(tc.tile_pool(name="pp_gate", bufs=1, space="PSUM"))

        # ---- constants ----
        adjT_sb = const_pool.tile([128, KT * N], f16)
        nc.sync.dma_start(adjT_sb[:], adjT_d[:])
        wgx4_sb = const_pool.tile([XR, BL * H], f16)
        nc.sync.dma_start(wgx4_sb[:], wgx4_d[:])
        wgh2_sb = const_pool.tile([128, 128], f16)
        nc.sync.dma_start(wgh2_sb[:], wgh2_d[:])
        wlblk_sb = const_pool.tile([128, 4 * 128], f16)
        nc.sync.dma_start(wlblk_sb[:], wlblk_d[:])
        wr12_sb = const_pool.tile([128, H // 2], f16)
        nc.sync.dma_start(wr12_sb[:], wr12_d[:])
        br1_sb = const_pool.tile([H // 2, 1], fp32)
        nc.sync.dma_start(br1_sb[:], br1_d[:])
        wr2_sb = const_pool.tile([H // 2, 1], f16)
        nc.sync.dma_start(wr2_sb[:], wr2_d[:])
        br2_sb = const_pool.tile([1, 1], fp32)
        nc.sync.dma_start(br2_sb[:], br2_d[:])

        # ---- persistent state (PP layout), zero-init ----
        hT2 = state_pool.tile([128, NPAIR * N], f16)
        cT2 = state_pool.tile([128, NPAIR * N], f16)
        nc.gpsimd.memset(hT2[:], 0.0)
        nc.gpsimd.memset(cT2[:], 0.0)

        # msg_ps is one shared tile [128, (pair, kt, qh)]: bank0 = pair0,
        # bank1 = pair1. Both banks' accumulation groups open at the single
        # M1x hoist point and each closes at its pair's M1h — the same group
        # pattern v3 ran safely.
        def emit_m1x(msg_ps, xc_tile, tl, pair):
            """x-part of M1 for one (timestep, pair) — no h dependency."""
            for kt in range(KT):
                nc.tensor.matmul(
                    msg_ps[:, pair * N + kt * 128: pair * N + (kt + 1) * 128],
                    xc_tile[:, tl * N + kt * 128: tl * N + (kt + 1) * 128],
                    wgx4_sb[:, pair * 128:(pair + 1) * 128],
                    # only the FIRST matmul in the PSUM bank may set
                    # start=True: it marks the whole 2KB bank pending-zero
                    start=(kt == 0), stop=False,
                )

        def get_xc(t):
            cidx, tl = divmod(t, TC)
            if cidx != get_xc.cidx:
                tcur = min(TC, t_steps - cidx * TC)
                xt = xc_pool.tile([XR, TC * N], f16, tag="xc")
                nc.sync.dma_start(
                    xt[:, 0:tcur * N],
                    xT4_d[:, cidx * TC:cidx * TC + tcur, :],
                )
                get_xc.cur = xt
                get_xc.cidx = cidx
            return get_xc.cur, tl
        get_xc.cidx = -1

        # prologue: x-part for t=0, both pairs
        xc_tile, tl0 = get_xc(0)
        msg_ps = pp_msg.tile([128, 2 * KT * 128], fp32, tag="msg")
        for pair in range(NPAIR):
            emit_m1x(msg_ps, xc_tile, tl0, pair)
        msg_ps_next = None

        for t in range(t_steps):
            for pair in range(NPAIR):
                # ---- M1 h-part: stationary = h state slice, moving =
                # block-diag Wgh [128, 128 (q' h')] ----
                for kt in range(KT):
                    nc.tensor.matmul(
                        msg_ps[:, pair * N + kt * 128: pair * N + (kt + 1) * 128],
                        hT2[:, pair * N + kt * 128: pair * N + (kt + 1) * 128],
                        wgh2_sb[:],
                        start=False, stop=(kt == KT - 1),
                    )

                # relu on DVE (PSUM -> SBUF; Pool cannot touch PSUM)
                msg_sb = msgsb_pool.tile([128, KT * 128], f16, tag="msg_sb")
                nc.vector.tensor_scalar_max(
                    msg_sb[:], msg_ps[:, pair * N:(pair + 1) * N], 0.0)

                # ---- M2: aggT = msg^T @ adjT  (PP out) ----
                agg_ps = pp_agg.tile([128, N], fp32, tag="agg")
                for kt in range(KT):
                    nc.tensor.matmul(
                        agg_ps[:],
                        msg_sb[:, kt * 128:(kt + 1) * 128],
                        adjT_sb[:, kt * N:(kt + 1) * N],
                        start=(kt == 0), stop=(kt == KT - 1),
                    )
                agg_sb = aggsb_pool.tile([128, N], f16, tag="agg_sb")
                nc.vector.tensor_copy(agg_sb[:], agg_ps[:])

                # ---- M3: gates via block-diag Wl, one matmul per gate ----
                gate_ps = pp_gate.tile([128, 4 * N], fp32, tag="gate")
                for g in range(4):
                    nc.tensor.matmul(
                        gate_ps[:, g * N:(g + 1) * N],
                        wlblk_sb[:, g * 128:(g + 1) * 128],
                        agg_sb[:],
                        start=True, stop=True,
                    )

                # next step's x-part (both pairs, single hoist point at the
                # end of pair1's matmuls): keeps the PE busy during the tail
                if pair == NPAIR - 1 and t + 1 < t_steps:
                    xc_tile, tl = get_xc(t + 1)
                    msg_ps_next = pp_msg.tile([128, 2 * KT * 128], fp32,
                                              tag="msg")
                    for pr in range(NPAIR):
                        emit_m1x(msg_ps_next, xc_tile, tl, pr)

                # ---- activations (gate order i, f, o, g); one sigmoid
                # over i,f,o cuts ACT instruction overhead ----
                sig = gact_pool.tile([128, 3 * N], f16, tag="sig")
                nc.scalar.activation(sig[:], gate_ps[:, 0:3 * N], AF.Sigmoid)
                tng = gact_pool.tile([128, N], f16, tag="tng")
                nc.scalar.activation(tng[:], gate_ps[:, 3 * N:4 * N], AF.Tanh)

                # ---- LSTM state update (all [128, 512], PP-aligned, fp16) ----
                c_sl = cT2[:, pair * N:(pair + 1) * N]
                t1 = tmp_pool.tile([128, N], f16, tag="t1")
                nc.vector.tensor_mul(t1[:], sig[:, N:2 * N], c_sl)
                t2 = tmp_pool.tile([128, N], f16, tag="t2")
                nc.vector.tensor_mul(t2[:], sig[:, 0:N], tng[:])
                nc.vector.tensor_add(c_sl, t1[:], t2[:])
                tc_ = tmp_pool.tile([128, N], f16, tag="tc")
                nc.scalar.activation(tc_[:], c_sl, AF.Tanh)
                nc.vector.tensor_mul(hT2[:, pair * N:(pair + 1) * N],
                                     sig[:, 2 * N:3 * N], tc_[:])

            msg_ps = msg_ps_next

        # ---- readout head: feat = h[:, station 0, :] ----
        r1_ps = pp_agg.tile([H // 2, BL], fp32, tag="agg")
        for b in range(BL):
            pair, q = divmod(b, 2)
            nc.tensor.matmul(
                r1_ps[:, b:b + 1],
                wr12_sb[q * H:(q + 1) * H, :],
                hT2[q * H:(q + 1) * H, pair * N: pair * N + 1],
                start=True, stop=True,
            )
        r1_sb = tmp_pool.tile([H // 2, BL], f16, tag="r1")
        nc.scalar.activation(r1_sb[:], r1_ps[:], AF.Relu, bias=br1_sb[:])
        r2_ps = pp_agg.tile([1, BL], fp32, tag="agg")
        nc.tensor.matmul(r2_ps[:], wr2_sb[:], r1_sb[:], start=True, stop=True)
        out_sb = tmp_pool.tile([1, BL], fp32, tag="out")
        nc.scalar.activation(out_sb[:], r2_ps[:], AF.Identity, bias=br2_sb[:])
        nc.sync.dma_start(out_d[:], out_sb[:])

    nc.compile()
    return nc


def prep_inputs(pollution_seq, adj, Wg, bg, Wl, bl, Wr1, br1, Wr2, br2,
                t_steps: int = T):
    """Host-side prep: shard + relayout. Returns per-core in_maps list."""
    assert np.allclose(bl, 0.0), "kernel folds bl only for bl==0"
    import ml_dtypes
    f32, f16 = np.float32, np.float16
    f8 = ml_dtypes.float8_e4m3

    ADJ_SCALE = 1.0
    adjT = np.ascontiguousarray(adj.T).astype(f32)               # [n, m]
    adjT_tiled = np.ascontiguousarray(
        adjT.reshape(KT, 128, N).transpose(1, 0, 2).reshape(128, KT * N)
    ).astype(f16)

    # wgx_blk4 [36, 256]: [(b,f), (b',h)] = Wg[f,h]*[b==b'] (+ bg on ones row)
    wgx9 = np.concatenate([Wg[:F], bg[None, :]], axis=0).astype(f32)   # [9, 64]
    wgx4 = np.zeros((XR, BL * H), f32)
    for b in range(BL):
        wgx4[b * FR:(b + 1) * FR, b * H:(b + 1) * H] = wgx9
    wgx4 = wgx4.astype(f16)

    # wgh2_blk [128, 128]: [(q,h), (q',h')] = Wg[F+h, h']*[q==q']
    wgh = Wg[F:].astype(f32)                                           # [64, 64]
    wgh2 = np.zeros((128, 128), f32)
    wgh2[:H, :H] = wgh
    wgh2[H:, H:] = wgh
    wgh2 = wgh2.astype(f16)

    # gate order (i, f, g, o) -> (i, f, o, g); block-diag per gate.
    # The 1/ADJ_SCALE undoes the adj fp8 pre-scale (agg arrives scaled).
    Wl_r = np.concatenate(
        [Wl[:, 0:H], Wl[:, H:2 * H], Wl[:, 3 * H:4 * H], Wl[:, 2 * H:3 * H]],
        axis=1).astype(f32) / ADJ_SCALE                                # [64, 256]
    wlblk = np.zeros((128, 4 * 128), f32)
    for g in range(4):
        wlblk[:H, g * 128:g * 128 + H] = Wl_r[:, g * H:(g + 1) * H]
        wlblk[H:, g * 128 + H:(g + 1) * 128] = Wl_r[:, g * H:(g + 1) * H]
    wlblk = wlblk.astype(f16)

    wr12 = np.concatenate([Wr1, Wr1], axis=0).astype(f16)              # [128, 32]

    common = dict(
        adjT=adjT_tiled, wgx4=wgx4, wgh2b=wgh2, wlblk=wlblk, wr12=wr12,
        br1=br1.reshape(H // 2, 1).astype(f32),
        wr2=Wr2.reshape(H // 2, 1).astype(f16),
        br2=br2.reshape(1, 1).astype(f32),
    )

    in_maps = []
    for i in range(NCORES):
        xc = pollution_seq[i * BL:(i + 1) * BL, :, :t_steps, :]   # [4, 512, t, 8]
        # xT4[b*9+f, t, n] = x[b, n, t, f]; row b*9+8 = ones
        xT4 = np.empty((XR, t_steps, N), f32)
        xT = xc.transpose(0, 3, 2, 1)                             # [4, 8, t, 512]
        for b in range(BL):
            xT4[b * FR:b * FR + F] = xT[b]
            xT4[b * FR + F] = 1.0
        m = dict(common)
        m["xT4"] = np.ascontiguousarray(xT4.astype(f16))
        in_maps.append(m)
    return in_maps


_NC_CACHE = {}


def _get_program(t_steps):
    if t_steps not in _NC_CACHE:
        _NC_CACHE[t_steps] = build_program(t_steps)
    return _NC_CACHE[t_steps]


def kernel(pollution_seq, adj, Wg, bg, Wl, bl, Wr1, br1, Wr2, br2,
           trace=False):
    if trace:
        _ensure_ntff_hook()
    nc = _get_program(T)
    in_maps = prep_inputs(pollution_seq, adj, Wg, bg, Wl, bl, Wr1, br1,
                          Wr2, br2, T)
    res = run_bass_kernel_spmd(nc, in_maps, list(range(NCORES)), trace=trace)
    outs = [res.results[i]["out"].reshape(1, BL) for i in range(NCORES)]
    full = np.concatenate([o.T for o in outs], axis=0).astype(np.float32)  # [32,1]
    if trace:
        kernel.last_exec_time_ns = res.exec_time_ns
        kernel.last_results = res
    return full


# revision 32
# speedup vs baseline: 1.1359x; 1.1359x over previous
"""GC-LSTM (graph-conv LSTM) Trainium2 kernel, v2.

Model (per batch b, timestep t):
    msg  = relu([x_t, h] @ Wg + bg)          # [N, H]
    agg  = adj @ msg                         # graph aggregation over stations
    gates= agg @ Wl + bl                     # [N, 4H]
    c    = sig(f)*c + sig(i)*tanh(g)
    h    = sig(o)*tanh(c)
final: out = relu(h[:, 0, :] @ Wr1 + br1) @ Wr2 + br2     # [B, 1]

Sharding: data-parallel over B=32 across 8 cores (B_local=4). adj and
weights replicated. h/c stay on-core through the whole T=168 scan.

Per-core layouts (b = 2*pair + q):
  SL (station-layout): partition = station-within-ktile.
    msg_ps  [128, (kt,b,h)]   PSUM, col = kt*256 + b*64 + h
    msg_sb  [128, (pair,kt,qh)] SBUF, col = pair*512 + kt*128 + q*64 + h
    adjT_sb [128, (kt,m)]     adjT_sb[p, kt*512+m] = adj[m, kt*128+p]
  PP (parity-paired feature-layout): partition = (q, h), free = (pair, m).
    hT2, cT2 [128, 1024] fp16
    agg_ps/gate_ps per pair: [128 (q,h'), m] with gate order (i, f, o, g)

Key tricks vs v1:
  - batch lives in the matmul CONTRACTION dim with block-diagonal weights
    (wgx_blk4 [36,256] for 4 batches; wgh2_blk / wl_blk [128,128] for the
    2 parities), collapsing many tiny matmuls into few wide ones.
  - relu and agg fp32->fp16 casts run on the idle Pool (gpsimd) engine.
  - c state is fp16 so every LSTM element-wise op is 16-bit SBUF-only
    (DVE 2x/4x perf modes).
"""

import os
import sys
import types
from contextlib import ExitStack

import numpy as np

import concourse.bass as bass
import concourse.mybir as mybir
import concourse.tile as tile
from concourse import bacc
from concourse.bass_utils import run_bass_kernel_spmd


def _ensure_ntff_hook():
    """Register the axon NTFF profile hook if the image's antenv lacks it."""
    try:
        from antenv import axon_hooks  # noqa: F401
        return
    except ImportError:
        pass
    hook = None
    try:
        import importlib
        tb = importlib.import_module("trn_agent_boot.trn_boot")
        hook = tb._ntff_profile_via_ctypes("/opt/axon/libaxon_pjrt.so")
    except Exception:
        hook = None
    mod = types.ModuleType("antenv.axon_hooks")
    mod._hook = hook
    mod.set_axon_ntff_profile_hook = lambda h: setattr(mod, "_hook", h)
    mod.get_axon_ntff_profile_hook = lambda: mod._hook
    import antenv
    sys.modules["antenv.axon_hooks"] = mod
    antenv.axon_hooks = mod

AF = mybir.ActivationFunctionType
DT = mybir.dt

# Problem constants (hardcoded per contract)
B, N, T, F = 32, 512, 168, 8
H = 64
NCORES = 8
BL = B // NCORES          # 4 batches per core
NPAIR = BL // 2           # 2 pairs
KT = N // 128             # 4 station k-tiles
FR = F + 1                # x rows incl. ones row
XR = BL * FR              # 36 x rows for all 4 batches
TC = 24                   # x prefetch chunk (168 = 7*24)


def build_program(t_steps: int = T, n_devices: int = NCORES):
    """Build the SPMD Bass program for one core (replicated on all cores)."""
    nc = bacc.Bacc("TRN2", target_bir_lowering=False, debug=False,
                   num_devices=n_devices)

    fp32 = DT.float32
    f16 = DT.float16
    f8 = DT.float8e4

    # DRAM I/O (per-core values supplied via in_maps)
    xT4_d = nc.dram_tensor("xT4", [XR, t_steps, N], f16, kind="ExternalInput").ap()
    adjT_d = nc.dram_tensor("adjT", [128, KT * N], f16, kind="ExternalInput").ap()
    wgx4_d = nc.dram_tensor("wgx4", [XR, BL * H], f16, kind="ExternalInput").ap()
    wgh2_d = nc.dram_tensor("wgh2b", [128, 128], f16, kind="ExternalInput").ap()
    wlblk_d = nc.dram_tensor("wlblk", [128, 4 * 128], f16, kind="ExternalInput").ap()
    wr12_d = nc.dram_tensor("wr12", [128, H // 2], f16, kind="ExternalInput").ap()
    br1_d = nc.dram_tensor("br1", [H // 2, 1], fp32, kind="ExternalInput").ap()
    wr2_d = nc.dram_tensor("wr2", [H // 2, 1], f16, kind="ExternalInput").ap()
    br2_d = nc.dram_tensor("br2", [1, 1], fp32, kind="ExternalInput").ap()
    out_d = nc.dram_tensor("out", [1, BL], fp32, kind="ExternalOutput").ap()

    with tile.TileContext(nc) as tc, ExitStack() as ctx:
        const_pool = ctx.enter_context(tc.tile_pool(name="const", bufs=1))
        state_pool = ctx.enter_context(tc.tile_pool(name="state", bufs=1))
        xc_pool = ctx.enter_context(tc.tile_pool(name="xc", bufs=2))
        msgsb_pool = ctx.enter_context(tc.tile_pool(name="msgsb", bufs=2))
        aggsb_pool = ctx.enter_context(tc.tile_pool(name="aggsb", bufs=3))
        gact_pool = ctx.enter_context(tc.tile_pool(name="gact", bufs=3))
        tmp_pool = ctx.enter_context(tc.tile_pool(name="tmp", bufs=6))

        pp_msg = ctx.enter_context(tc.tile_pool(name="pp_msg", bufs=1, space="PSUM"))
        pp_agg = ctx.enter_context(tc.tile_pool(name="pp_agg", bufs=2, space="PSUM"))
        pp_gate = ctx.enter_context(tc.tile_pool(name="pp_gate", bufs=1, space="PSUM"))

        # ---- constants ----
        adjT_sb = const_pool.tile([128, KT * N], f16)
        nc.sync.dma_start(adjT_sb[:], adjT_d[:])
        wgx4_sb = const_pool.tile([XR, BL * H], f16)
        nc.sync.dma_start(wgx4_sb[:], wgx4_d[:])
        wgh2_sb = const_pool.tile([128, 128], f16)
        nc.sync.dma_start(wgh2_sb[:], wgh2_d[:])
        wlblk_sb = const_pool.tile([128, 4 * 128], f16)
        nc.sync.dma_start(wlblk_sb[:], wlblk_d[:])
        wr12_sb = const_pool.tile([128, H // 2], f16)
        nc.sync.dma_start(wr12_sb[:], wr12_d[:])
        br1_sb = const_pool.tile([H // 2, 1], fp32)
        nc.sync.dma_start(br1_sb[:], br1_d[:])
        wr2_sb = const_pool.tile([H // 2, 1], f16)
        nc.sync.dma_start(wr2_sb[:], wr2_d[:])
        br2_sb = const_pool.tile([1, 1], fp32)
        nc.sync.dma_start(br2_sb[:], br2_d[:])

        # ---- persistent state (PP layout), zero-init ----
        hT2 = state_pool.tile([128, NPAIR * N], f16)
        cT2 = state_pool.tile([128, NPAIR * N], f16)
        nc.gpsimd.memset(hT2[:], 0.0)
        nc.gpsimd.memset(cT2[:], 0.0)

        # msg_ps is one shared tile [128, (pair, kt, qh)]: bank0 = pair0,
        # bank1 = pair1. Both banks' accumulation groups open at the single
        # M1x hoist point and each closes at its pair's M1h — the same group
        # pattern v3 ran safely.
        def emit_m1x(msg_ps, xc_tile, tl, pair):
            """x-part of M1 for one (timestep, pair) — no h dependency."""
            for kt in range(KT):
                nc.tensor.matmul(
                    msg_ps[:, pair * N + kt * 128: pair * N + (kt + 1) * 128],
                    xc_tile[:, tl * N + kt * 128: tl * N + (kt + 1) * 128],
                    wgx4_sb[:, pair * 128:(pair + 1) * 128],
                    # only the FIRST matmul in the PSUM bank may set
                    # start=True: it marks the whole 2KB bank pending-zero
                    start=(kt == 0), stop=False,
                )

        def get_xc(t):
            cidx, tl = divmod(t, TC)
            if cidx != get_xc.cidx:
                tcur = min(TC, t_steps - cidx * TC)
                xt = xc_pool.tile([XR, TC * N], f16, tag="xc")
                nc.sync.dma_start(
                    xt[:, 0:tcur * N],
                    xT4_d[:, cidx * TC:cidx * TC + tcur, :],
                )
                get_xc.cur = xt
                get_xc.cidx = cidx
            return get_xc.cur, tl
        get_xc.cidx = -1

        # prologue: x-part for t=0, both pairs
        xc_tile, tl0 = get_xc(0)
        msg_ps = pp_msg.tile([128, 2 * KT * 128], fp32, tag="msg")
        for pair in range(NPAIR):
            emit_m1x(msg_ps, xc_tile, tl0, pair)
        msg_ps_next = None

        for t in range(t_steps):
            for pair in range(NPAIR):
                # ---- M1 h-part: stationary = h state slice, moving =
                # block-diag Wgh [128, 128 (q' h')] ----
                for kt in range(KT):
                    nc.tensor.matmul(
                        msg_ps[:, pair * N + kt * 128: pair * N + (kt + 1) * 128],
                        hT2[:, pair * N + kt * 128: pair * N + (kt + 1) * 128],
                        wgh2_sb[:],
                        start=False, stop=(kt == KT - 1),
                    )

                # relu on DVE (PSUM -> SBUF; Pool cannot touch PSUM)
                msg_sb = msgsb_pool.tile([128, KT * 128], f16, tag="msg_sb")
                nc.vector.tensor_scalar_max(
                    msg_sb[:], msg_ps[:, pair * N:(pair + 1) * N], 0.0)

                # ---- M2: aggT = msg^T @ adjT  (PP out) ----
                agg_ps = pp_agg.tile([128, N], fp32, tag="agg")
                for kt in range(KT):
                    nc.tensor.matmul(
                        agg_ps[:],
                        msg_sb[:, kt * 128:(kt + 1) * 128],
                        adjT_sb[:, kt * N:(kt + 1) * N],
                        start=(kt == 0), stop=(kt == KT - 1),
                    )
                agg_sb = aggsb_pool.tile([128, N], f16, tag="agg_sb")
                nc.vector.tensor_copy(agg_sb[:], agg_ps[:])

                # ---- M3: gates via block-diag Wl, one matmul per gate ----
                gate_ps = pp_gate.tile([128, 4 * N], fp32, tag="gate")
                for g in range(4):
                    nc.tensor.matmul(
                        gate_ps[:, g * N:(g + 1) * N],
                        wlblk_sb[:, g * 128:(g + 1) * 128],
                        agg_sb[:],
                        start=True, stop=True,
                    )

                # next step's x-part (both pairs, single hoist point at the
                # end of pair1's matmuls): keeps the PE busy during the tail
                if pair == NPAIR - 1 and t + 1 < t_steps:
                    xc_tile, tl = get_xc(t + 1)
                    msg_ps_next = pp_msg.tile([128, 2 * KT * 128], fp32,
                                              tag="msg")
                    for pr in range(NPAIR):
                        emit_m1x(msg_ps_next, xc_tile, tl, pr)

                # ---- activations (gate order i, f, o, g) ----
                sig = gact_pool.tile([128, 3 * N], f16, tag="sig")
                nc.scalar.activation(sig[:, 0:2 * N], gate_ps[:, 0:2 * N],
                                     AF.Sigmoid)
                tng = gact_pool.tile([128, N], f16, tag="tng")
                nc.scalar.activation(tng[:], gate_ps[:, 3 * N:4 * N], AF.Tanh)
                nc.scalar.activation(sig[:, 2 * N:3 * N], gate_ps[:, 2 * N:3 * N],
                                     AF.Sigmoid)

                # ---- LSTM state update (all [128, 512], PP-aligned, fp16) ----
                c_sl = cT2[:, pair * N:(pair + 1) * N]
                t1 = tmp_pool.tile([128, N], f16, tag="t1")
                nc.vector.tensor_mul(t1[:], sig[:, N:2 * N], c_sl)
                t2 = tmp_pool.tile([128, N], f16, tag="t2")
                nc.vector.tensor_mul(t2[:], sig[:, 0:N], tng[:])
                nc.vector.tensor_add(c_sl, t1[:], t2[:])
                tc_ = tmp_pool.tile([128, N], f16, tag="tc")
                nc.scalar.activation(tc_[:], c_sl, AF.Tanh)
                nc.vector.tensor_mul(hT2[:, pair * N:(pair + 1) * N],
                                     sig[:, 2 * N:3 * N], tc_[:])

            msg_ps = msg_ps_next

        # ---- readout head: feat = h[:, station 0, :] ----
        r1_ps = pp_agg.tile([H // 2, BL], fp32, tag="agg")
        for b in range(BL):
            pair, q = divmod(b, 2)
            nc.tensor.matmul(
                r1_ps[:, b:b + 1],
                wr12_sb[q * H:(q + 1) * H, :],
                hT2[q * H:(q + 1) * H, pair * N: pair * N + 1],
                start=True, stop=True,
            )
        r1_sb = tmp_pool.tile([H // 2, BL], f16, tag="r1")
        nc.scalar.activation(r1_sb[:], r1_ps[:], AF.Relu, bias=br1_sb[:])
        r2_ps = pp_agg.tile([1, BL], fp32, tag="agg")
        nc.tensor.matmul(r2_ps[:], wr2_sb[:], r1_sb[:], start=True, stop=True)
        out_sb = tmp_pool.tile([1, BL], fp32, tag="out")
        nc.scalar.activation(out_sb[:], r2_ps[:], AF.Identity, bias=br2_sb[:])
        nc.sync.dma_start(out_d[:], out_sb[:])

    nc.compile()
    return nc


def prep_inputs(pollution_seq, adj, Wg, bg, Wl, bl, Wr1, br1, Wr2, br2,
                t_steps: int = T):
    """Host-side prep: shard + relayout. Returns per-core in_maps list."""
    assert np.allclose(bl, 0.0), "kernel folds bl only for bl==0"
    import ml_dtypes
    f32, f16 = np.float32, np.float16
    f8 = ml_dtypes.float8_e4m3

    ADJ_SCALE = 1.0
    adjT = np.ascontiguousarray(adj.T).astype(f32)               # [n, m]
    adjT_tiled = np.ascontiguousarray(
        adjT.reshape(KT, 128, N).transpose(1, 0, 2).reshape(128, KT * N)
    ).astype(f16)

    # wgx_blk4 [36, 256]: [(b,f), (b',h)] = Wg[f,h]*[b==b'] (+ bg on ones row)
    wgx9 = np.concatenate([Wg[:F], bg[None, :]], axis=0).astype(f32)   # [9, 64]
    wgx4 = np.zeros((XR, BL * H), f32)
    for b in range(BL):
        wgx4[b * FR:(b + 1) * FR, b * H:(b + 1) * H] = wgx9
    wgx4 = wgx4.astype(f16)

    # wgh2_blk [128, 128]: [(q,h), (q',h')] = Wg[F+h, h']*[q==q']
    wgh = Wg[F:].astype(f32)                                           # [64, 64]
    wgh2 = np.zeros((128, 128), f32)
    wgh2[:H, :H] = wgh
    wgh2[H:, H:] = wgh
    wgh2 = wgh2.astype(f16)

    # gate order (i, f, g, o) -> (i, f, o, g); block-diag per gate.
    # The 1/ADJ_SCALE undoes the adj fp8 pre-scale (agg arrives scaled).
    Wl_r = np.concatenate(
        [Wl[:, 0:H], Wl[:, H:2 * H], Wl[:, 3 * H:4 * H], Wl[:, 2 * H:3 * H]],
        axis=1).astype(f32) / ADJ_SCALE                                # [64, 256]
    wlblk = np.zeros((128, 4 * 128), f32)
    for g in range(4):
        wlblk[:H, g * 128:g * 128 + H] = Wl_r[:, g * H:(g + 1) * H]
        wlblk[H:, g * 128 + H:(g + 1) * 128] = Wl_r[:, g * H:(g + 1) * H]
    wlblk = wlblk.astype(f16)

    wr12 = np.concatenate([Wr1, Wr1], axis=0).astype(f16)              # [128, 32]

    common = dict(
        adjT=adjT_tiled, wgx4=wgx4, wgh2b=wgh2, wlblk=wlblk, wr12=wr12,
        br1=br1.reshape(H // 2, 1).astype(f32),
        wr2=Wr2.reshape(H // 2, 1).astype(f16),
        br2=br2.reshape(1, 1).astype(f32),
    )

    in_maps = []
    for i in range(NCORES):
        xc = pollution_seq[i * BL:(i + 1) * BL, :, :t_steps, :]   # [4, 512, t, 8]
        # xT4[b*9+f, t, n] = x[b, n, t, f]; row b*9+8 = ones
        xT4 = np.empty((XR, t_steps, N), f32)
        xT = xc.transpose(0, 3, 2, 1)                             # [4, 8, t, 512]
        for b in range(BL):
            xT4[b * FR:b * FR + F] = xT[b]
            xT4[b * FR + F] = 1.0
        m = dict(common)
        m["xT4"] = np.ascontiguousarray(xT4.astype(f16))
        in_maps.append(m)
    return in_maps


_NC_CACHE = {}


def _get_program(t_steps):
    if t_steps not in _NC_CACHE:
        _NC_CACHE[t_steps] = build_program(t_steps)
    return _NC_CACHE[t_steps]


def kernel(pollution_seq, adj, Wg, bg, Wl, bl, Wr1, br1, Wr2, br2,
           trace=False):
    if trace:
        _ensure_ntff_hook()
    nc = _get_program(T)
    in_maps = prep_inputs(pollution_seq, adj, Wg, bg, Wl, bl, Wr1, br1,
                          Wr2, br2, T)
    res = run_bass_kernel_spmd(nc, in_maps, list(range(NCORES)), trace=trace)
    outs = [res.results[i]["out"].reshape(1, BL) for i in range(NCORES)]
    full = np.concatenate([o.T for o in outs], axis=0).astype(np.float32)  # [32,1]
    if trace:
        kernel.last_exec_time_ns = res.exec_time_ns
        kernel.last_results = res
    return full
